# revision 4
# baseline (speedup 1.0000x reference)
"""Two-layer GAT (PyG GATConv, H=4 heads, C=32) on 8 Trainium2 NeuronCores.

Strategy:
  - Host: degree-balanced 1D partition of nodes into 128-node blocks (98
    blocks/core), edges bucketed to the block owning their dst. Per-block
    edge grids [128, T] with per-edge src row ids (for indirect gather) and
    dst slot ids (for one-hot build).
  - Device, per layer: node transform (x @ [W.T | v_src | v_dst]) producing
    per-node rows [xp(128) | a_src(4) | a_dst(4)]; AllGather of the shard;
    per-block edge phase: indirect-gather of per-edge rows, one-hot S from
    iota==dst, adst broadcast via S transpose + tiny matmul, leaky-relu+exp,
    aggregation + softmax denominator in a single PE matmul per tile,
    post-division by the denominator, bias (+relu on layer 1). Layer-2 node
    transform is fused into the layer-1 edge phase per block.
"""
import sys

sys.path.insert(0, "/opt/trn_rl_repo")

import math
import heapq
import numpy as np

import concourse.bass as bass
import concourse.bacc as bacc_mod
import concourse.mybir as mybir
import concourse.tile as tile
from concourse.masks import make_identity
from concourse.bass_utils import run_bass_kernel_spmd

F32 = mybir.dt.float32
I32 = mybir.dt.int32

H = 4
C = 32
HC = H * C          # 128
SLOPE = 0.2
P = 128             # partitions / block size
NCORES = 8
PAD_DST = 999.0     # dst slot sentinel for padded edges


# ----------------------------------------------------------------------------
# host-side graph partitioning + input prep
# ----------------------------------------------------------------------------

def host_prep(x, edge_index, W0, as0, ad0, b0, W1, as1, ad1, b1):
    N, F_IN = x.shape
    E = edge_index.shape[1]
    src = edge_index[0].astype(np.int64)
    dst = edge_index[1].astype(np.int64)

    NBC = math.ceil(N / (P * NCORES))       # blocks per core
    NB = NBC * NCORES
    NPADC = NBC * P                         # padded nodes per core

    # --- degree-balanced assignment of nodes to blocks (cap 128 nodes) ---
    deg = np.bincount(dst, minlength=N).astype(np.int64)
    order = np.argsort(-deg, kind="stable")
    nodeblock = np.empty(N, np.int32)
    slot = np.empty(N, np.int32)
    heap = [(0, b) for b in range(NB)]
    heapq.heapify(heap)
    counts = np.zeros(NB, np.int32)
    for n in order:
        s, b = heapq.heappop(heap)
        nodeblock[n] = b
        slot[n] = counts[b]
        counts[b] += 1
        if counts[b] < P:
            heapq.heappush(heap, (s + int(deg[n]), b))
    rowof = (nodeblock.astype(np.int64) * P + slot).astype(np.int32)  # global padded row

    # --- edge bucketing by dst block ---
    eb = nodeblock[dst]                       # block of each edge
    eorder = np.argsort(eb, kind="stable")
    ebs = eb[eorder]
    bcounts = np.bincount(eb, minlength=NB)
    starts = np.zeros(NB + 1, np.int64)
    np.cumsum(bcounts, out=starts[1:])
    pos_sorted = np.arange(E, dtype=np.int64) - starts[ebs]
    pos = np.empty(E, np.int64)
    pos[eorder] = pos_sorted
    T = max(1, math.ceil(int(bcounts.max()) / P))

    srcidx_all = np.zeros((NB, P, T), np.int32)
    dstf_all = np.full((NB, P, T), PAD_DST, np.float32)
    pp = (pos % P).astype(np.int64)
    jj = (pos // P).astype(np.int64)
    srcidx_all[eb, pp, jj] = rowof[src]
    dstf_all[eb, pp, jj] = (rowof[dst] % P).astype(np.float32)

    # --- per-core transposed node features (padded) ---
    nodeperm = np.full(NB * P, -1, np.int64)
    nodeperm[rowof] = np.arange(N)
    xcs = []
    for c in range(NCORES):
        rows = nodeperm[c * NPADC:(c + 1) * NPADC]
        xc = np.zeros((NPADC, F_IN), np.float32)
        valid = rows >= 0
        xc[valid] = x[rows[valid]]
        xcs.append(np.ascontiguousarray(xc.T))  # [F_IN, NPADC]

    # --- combined weights: [W.T | v_src | v_dst] ---
    def comb(W, asv, adv):
        F = W.shape[1]
        vs = np.einsum("hcf,hc->fh", W.reshape(H, C, F), asv)
        vd = np.einsum("hcf,hc->fh", W.reshape(H, C, F), adv)
        return np.ascontiguousarray(
            np.concatenate([W.T, vs, vd], axis=1).astype(np.float32))

    Wc0 = comb(W0, as0, ad0)                  # [F_IN, HC+8]
    Wc1 = comb(W1, as1, ad1)                  # [HC, HC+8]
    iota = np.tile(np.arange(P, dtype=np.float32), (P, 1))
    b0b = np.tile(b0.astype(np.float32), (P, 1))
    b1b = np.tile(b1.astype(np.float32), (P, 1))

    in_maps = []
    for c in range(NCORES):
        in_maps.append({
            "xT": xcs[c],
            "Wc0": Wc0,
            "Wc1": Wc1,
            "b0b": b0b,
            "b1b": b1b,
            "iota": iota,
            "srcidx": np.ascontiguousarray(
                srcidx_all.reshape(NCORES, NBC, P, T)[c]),
            "dstf": np.ascontiguousarray(
                dstf_all.reshape(NCORES, NBC, P, T)[c]),
        })
    meta = dict(N=N, E=E, F_IN=F_IN, NBC=NBC, NB=NB, NPADC=NPADC, T=T,
                nodeperm=nodeperm, rowof=rowof)
    return in_maps, meta


# ----------------------------------------------------------------------------
# device program
# ----------------------------------------------------------------------------

def build_program(F_IN, NBC, T):
    NPADC = NBC * P
    NPAD = NPADC * NCORES
    D = HC + 8                      # row width: xp | a_src | a_dst
    FCH = F_IN // P                 # feature chunks for layer-1 node matmul
    assert F_IN % P == 0

    nc = bacc_mod.Bacc()
    xT = nc.declare_dram_parameter("xT", [F_IN, NPADC], F32, isOutput=False)
    Wc0 = nc.declare_dram_parameter("Wc0", [F_IN, D], F32, isOutput=False)
    Wc1 = nc.declare_dram_parameter("Wc1", [HC, D], F32, isOutput=False)
    b0b = nc.declare_dram_parameter("b0b", [P, HC], F32, isOutput=False)
    b1b = nc.declare_dram_parameter("b1b", [P, HC], F32, isOutput=False)
    iota = nc.declare_dram_parameter("iota", [P, P], F32, isOutput=False)
    srcidx = nc.declare_dram_parameter("srcidx", [NBC, P, T], I32, isOutput=False)
    dstf = nc.declare_dram_parameter("dstf", [NBC, P, T], F32, isOutput=False)
    out = nc.declare_dram_parameter("out", [NPADC, HC], F32, isOutput=True)

    F0_loc = nc.dram_tensor("F0_loc", [NPADC, D], F32)
    F1_loc = nc.dram_tensor("F1_loc", [NPADC, D], F32)
    F0_all = nc.dram_tensor("F0_all", [NPAD, D], F32, addr_space="Shared")
    F1_all = nc.dram_tensor("F1_all", [NPAD, D], F32, addr_space="Shared")
    groups = [list(range(NCORES))]

    with tile.TileContext(nc) as tc, \
         tc.tile_pool(name="const", bufs=1) as cpool, \
         tc.tile_pool(name="work", bufs=3) as wp, \
         tc.tile_pool(name="gbuf", bufs=2) as gp, \
         tc.tile_pool(name="ps_out", bufs=2, space="PSUM") as ps_out, \
         tc.tile_pool(name="ps_st", bufs=2, space="PSUM") as ps_st, \
         tc.tile_pool(name="ps_f1", bufs=2, space="PSUM") as ps_f1:

        # ---- resident constants ----
        wc0_t = [cpool.tile([P, D], F32, tag=f"wc0_{k}", name=f"wc0_{k}") for k in range(FCH)]
        for k in range(FCH):
            nc.sync.dma_start(out=wc0_t[k][:], in_=Wc0[k * P:(k + 1) * P, :])
        wc1_t = cpool.tile([P, D], F32, tag="wc1")
        nc.sync.dma_start(out=wc1_t[:], in_=Wc1[:])
        b0_t = cpool.tile([P, HC], F32, tag="b0")
        nc.sync.dma_start(out=b0_t[:], in_=b0b[:])
        b1_t = cpool.tile([P, HC], F32, tag="b1")
        nc.sync.dma_start(out=b1_t[:], in_=b1b[:])
        iota_t = cpool.tile([P, P], F32, tag="iota")
        nc.sync.dma_start(out=iota_t[:], in_=iota[:])
        ident = cpool.tile([P, P], F32, tag="ident")
        make_identity(nc, ident[:])

        # ---- layer-1 node phase: F0_loc rows = [xp | a_src | a_dst] ----
        for i in range(NBC):
            xt = wp.tile([P, FCH, P], F32, tag="xt")
            nc.sync.dma_start(
                out=xt[:],
                in_=xT[:, i * P:(i + 1) * P].rearrange("(k p) n -> p k n", p=P))
            f0_ps = ps_f1.tile([P, D], F32, tag="f1ps")
            for k in range(FCH):
                nc.tensor.matmul(f0_ps[:], lhsT=xt[:, k, :], rhs=wc0_t[k][:],
                                 start=(k == 0), stop=(k == FCH - 1))
            f0_sb = wp.tile([P, D], F32, tag="f0sb")
            nc.scalar.copy(out=f0_sb[:], in_=f0_ps[:])
            nc.sync.dma_start(out=F0_loc[i * P:(i + 1) * P, :], in_=f0_sb[:])

        nc.gpsimd.collective_compute(
            "AllGather", mybir.AluOpType.bypass, replica_groups=groups,
            ins=[F0_loc[:]], outs=[F0_all[:]])

        # ---- edge phase (shared for both layers) ----
        def edge_phase(F_all_d, F_loc_d, layer):
            bias_t = b0_t if layer == 0 else b1_t
            for b in range(NBC):
                offs = wp.tile([P, T], I32, tag="offs")
                nc.sync.dma_start(out=offs[:], in_=srcidx[b])
                dsts = wp.tile([P, T], F32, tag="dsts")
                nc.sync.dma_start(out=dsts[:], in_=dstf[b])
                adst_b = wp.tile([P, 4], F32, tag="adstb")
                nc.sync.dma_start(
                    out=adst_b[:], in_=F_loc_d[b * P:(b + 1) * P, HC + 4:HC + 8])

                G = gp.tile([P, T, D], F32, tag="G")
                out_ps = ps_out.tile([P, HC + 4], F32, tag="ops")
                for j in range(T):
                    nc.gpsimd.indirect_dma_start(
                        out=G[:, j, :], out_offset=None, in_=F_all_d[:],
                        in_offset=bass.IndirectOffsetOnAxis(
                            ap=offs[:, j:j + 1], axis=0))
                    # S[e, d] = (iota == dst_slot[e])
                    S = wp.tile([P, P], F32, tag="S")
                    nc.vector.tensor_tensor(
                        out=S[:], in0=iota_t[:],
                        in1=dsts[:, j:j + 1].to_broadcast([P, P]),
                        op=mybir.AluOpType.is_equal)
                    # ST = S.T  (PE transpose via identity, then copy to SBUF)
                    st_ps = ps_st.tile([P, P], F32, tag="stps")
                    nc.tensor.transpose(out=st_ps[:], in_=S[:], identity=ident[:])
                    ST = wp.tile([P, P], F32, tag="ST")
                    nc.vector.tensor_copy(out=ST[:], in_=st_ps[:])
                    # adst per edge = S @ adst_b
                    adst_ps = ps_st.tile([P, 4], F32, tag="adps", bufs=1)
                    nc.tensor.matmul(adst_ps[:], lhsT=ST[:], rhs=adst_b[:],
                                     start=True, stop=True)
                    # e = a_src + a_dst ; lrelu ; exp (into G's a_src slots)
                    e_sb = wp.tile([P, 4], F32, tag="esb")
                    nc.vector.tensor_tensor(
                        out=e_sb[:], in0=G[:, j, HC:HC + 4], in1=adst_ps[:],
                        op=mybir.AluOpType.add)
                    e2_sb = wp.tile([P, 4], F32, tag="e2sb")
                    nc.vector.tensor_scalar(
                        out=e2_sb[:], in0=e_sb[:], scalar1=SLOPE, scalar2=None,
                        op0=mybir.AluOpType.mult)
                    nc.vector.tensor_tensor(
                        out=e2_sb[:], in0=e2_sb[:], in1=e_sb[:],
                        op=mybir.AluOpType.max)
                    nc.scalar.activation(
                        G[:, j, HC:HC + 4], e2_sb[:],
                        mybir.ActivationFunctionType.Exp)
                    # msg scale: xp *= ex (per head)
                    nc.vector.tensor_tensor(
                        out=G[:, j, 0:HC].rearrange("p (h c) -> p h c", h=H),
                        in0=G[:, j, 0:HC].rearrange("p (h c) -> p h c", h=H),
                        in1=G[:, j, HC:HC + 4][:, :, None].to_broadcast([P, H, C]),
                        op=mybir.AluOpType.mult)
                    # aggregate: out[d, 0:128] += sum_e S[e,d] * msg[e,:]
                    #            out[d, 128:132] += sum_e S[e,d] * ex[e,:]
                    nc.tensor.matmul(out_ps[:], lhsT=S[:], rhs=G[:, j, 0:HC + 4],
                                     start=(j == 0), stop=(j == T - 1))

                # normalize + bias (+relu)
                den = wp.tile([P, 4], F32, tag="den")
                nc.vector.tensor_scalar(
                    out=den[:], in0=out_ps[:, HC:HC + 4], scalar1=1e-16,
                    scalar2=None, op0=mybir.AluOpType.add)
                rec = wp.tile([P, 4], F32, tag="rec")
                nc.vector.reciprocal(rec[:], den[:])
                h_sb = wp.tile([P, HC], F32, tag="hsb")
                nc.vector.tensor_tensor(
                    out=h_sb[:].rearrange("p (h c) -> p h c", h=H),
                    in0=out_ps[:, 0:HC].rearrange("p (h c) -> p h c", h=H),
                    in1=rec[:][:, :, None].to_broadcast([P, H, C]),
                    op=mybir.AluOpType.mult)
                nc.vector.tensor_tensor(out=h_sb[:], in0=h_sb[:], in1=bias_t[:],
                                        op=mybir.AluOpType.add)
                if layer == 0:
                    nc.vector.tensor_scalar(
                        out=h_sb[:], in0=h_sb[:], scalar1=0.0, scalar2=None,
                        op0=mybir.AluOpType.max)
                    # fused layer-2 node transform for this block
                    hT_ps = ps_st.tile([P, P], F32, tag="htps", bufs=1)
                    nc.tensor.transpose(out=hT_ps[:], in_=h_sb[:],
                                        identity=ident[:])
                    hT_sb = wp.tile([P, P], F32, tag="htsb")
                    nc.scalar.copy(out=hT_sb[:], in_=hT_ps[:])
                    f1_ps = ps_f1.tile([P, D], F32, tag="f1ps")
                    nc.tensor.matmul(f1_ps[:], lhsT=hT_sb[:], rhs=wc1_t[:],
                                     start=True, stop=True)
                    f1_sb = wp.tile([P, D], F32, tag="f1sb")
                    nc.scalar.copy(out=f1_sb[:], in_=f1_ps[:])
                    nc.sync.dma_start(out=F1_loc[b * P:(b + 1) * P, :],
                                      in_=f1_sb[:])
                else:
                    nc.sync.dma_start(out=out[b * P:(b + 1) * P, :], in_=h_sb[:])

        edge_phase(F0_all, F0_loc, 0)
        nc.gpsimd.collective_compute(
            "AllGather", mybir.AluOpType.bypass, replica_groups=groups,
            ins=[F1_loc[:]], outs=[F1_all[:]])
        edge_phase(F1_all, F1_loc, 1)

    nc.compile()
    return nc


# ----------------------------------------------------------------------------
# public entry
# ----------------------------------------------------------------------------

_CACHE = {}


def kernel(x, edge_index, W0, att_src0, att_dst0, b0, W1, att_src1, att_dst1,
           b1, _want_results=False, _trace=False):
    x = np.asarray(x, np.float32)
    edge_index = np.asarray(edge_index)
    args = [np.asarray(a, np.float32) for a in
            (W0, att_src0, att_dst0, b0, W1, att_src1, att_dst1, b1)]

    in_maps, meta = host_prep(x, edge_index, *args)
    key = (x.shape, edge_index.shape[1], meta["T"])
    if key not in _CACHE:
        _CACHE[key] = build_program(meta["F_IN"], meta["NBC"], meta["T"])
    nc = _CACHE[key]

    res = run_bass_kernel_spmd(nc, in_maps, list(range(NCORES)), trace=_trace)

    NPADC = meta["NPADC"]
    nodeperm = meta["nodeperm"]
    out_full = np.empty((meta["N"], HC), np.float32)
    for c in range(NCORES):
        rows = nodeperm[c * NPADC:(c + 1) * NPADC]
        valid = rows >= 0
        out_full[rows[valid]] = res.results[c]["out"][valid]
    if _want_results:
        return out_full, res
    return out_full
